# revision 22
# baseline (speedup 1.0000x reference)
"""Trainium2 Bass kernel for DBRX attention (B=2, S=2048, D=4096, 32 q-heads,
8 kv-heads GQA, causal, RoPE), 8-way head-tensor-parallel over 8 cores.

Sharding: core c owns q-heads 4c..4c+3 and kv-head c (GQA groups stay
aligned). Each core computes its 512-dim slice of attention output, then a
full-token out_proj partial with its 512-row slice of out_w; the host sums
the 8 partials (the "all-reduce after out_proj" of the hint, done at gather
time).

All matmul operands are bf16 (fp32 PSUM accumulation); softmax/rope scalar
math stays fp32. Design points, tuned against the CoreSim timeline model:
  - QKV: feature-outer / contraction-inner with the whole 512-token hidden
    slab SBUF-resident, so each feature's 32 matmuls run back-to-back and
    evictions (clip + RoPE) overlap the next feature's matmuls. PSUM tags
    rotate so the PE never waits on eviction.
  - RoPE via a signed permutation matmul on the PE (rotate_half) + 3 DVE
    elementwise ops; q stays SBUF-resident for phase 2 (no DRAM spill).
  - attention with S computed TRANSPOSED (S_T[k,q] = k_T^T @ q_T) and the
    4 q-heads of the core FUSED into one 512-wide moving operand per
    128-token q-tile (GQA: they share K and V). Post-softmax P_T is already
    K-major for the P^T V matmul: no transposes.
  - softmax without a max pass: exp((S - C)/sqrt(d)) with constant C; exact
    for any C (softmax shift invariance), safe in fp32 for |S_scaled| < ~88.
  - k-blocks processed in pairs sharing one [128,2,512] PSUM tile so exp
    runs as a single wide Act op; row sums l via a ones-vector matmul
    (partition reduction); 1/l broadcast back with a rank-1 f32r matmul.
  - causal structure: strictly-upper k-blocks skipped, diagonal 128x128
    blocks masked with a host-provided additive tile (per head).
"""

import math
import os
import sys

import numpy as np
import ml_dtypes

for _p in ("/root/.axon_site/_ro/trn_rl_repo", "/opt/trn_rl_repo"):
    if os.path.isdir(_p) and _p not in sys.path:
        sys.path.append(_p)

import concourse.bass as bass
import concourse.tile as tile
from concourse import bacc, mybir
from concourse.bass_utils import run_bass_kernel_spmd

F32 = mybir.dt.float32
F32R = mybir.dt.float32r
BF16 = mybir.dt.bfloat16
NP_BF16 = ml_dtypes.bfloat16


def R(ap):
    return ap


def _tri_bcast(tri):
    """[128,4,128] view of the [128,128] tri tile, stride-0 over heads."""
    try:
        return tri[:].unsqueeze(1).broadcast_to((128, 4, 128))
    except Exception:
        return None

N_CORES = 8
DH = 128          # head dim
HPC = 4           # q heads per core
NF = HPC + 2      # qkv feature tiles of 128 per core (4 q + 1 k + 1 v)
CLIP = 8.0
ROPE_THETA = 500000.0
ISQ = 1.0 / math.sqrt(DH)
EXP_C = 10.0      # constant softmax shift (exact for any value; see header)
NEG = -1.0e38


def build_program(B, S, D, causal=True, debug=False, reps=1):
    """Build the single-core Bass program (same program on all 8 cores)."""
    T = B * S                  # total tokens
    KB = D // 128              # contraction chunks for the projections
    SKB = S // 128             # k blocks per batch in attention
    QT = S // 128              # q tiles per batch in attention
    TT = 512                   # token tile for phase 1
    PW = 1024                  # out_proj psum group + DMA width (2 banks)

    nc = bacc.Bacc(
        "TRN2",
        target_bir_lowering=False,
        debug=debug,
        num_devices=N_CORES,
    )

    hid = nc.dram_tensor("hidden_t", [D, T], BF16, kind="ExternalInput")
    wqkv = nc.dram_tensor("wqkv_t", [D, NF * 128], BF16, kind="ExternalInput")
    outw = nc.dram_tensor("outw_t", [HPC * DH, D], BF16, kind="ExternalInput")
    cs_d = nc.dram_tensor("cs_t", [2, DH, T], F32, kind="ExternalInput")
    rot_d = nc.dram_tensor("rot_t", [DH, DH], BF16, kind="ExternalInput")
    tri_d = nc.dram_tensor("trimask", [128, 128], F32, kind="ExternalInput")
    idn_d = nc.dram_tensor("identity", [128, 128], BF16, kind="ExternalInput")
    out_d = nc.dram_tensor("out_partial", [T // 128, D // PW, 128, PW], F32,
                           kind="ExternalOutput")

    Exp = mybir.ActivationFunctionType.Exp
    Copy = mybir.ActivationFunctionType.Copy
    Alu = mybir.AluOpType

    from contextlib import ExitStack

    with ExitStack() as ctx:
        tc = ctx.enter_context(tile.TileContext(nc))
        PSUM = bass.MemorySpace.PSUM
        constp = ctx.enter_context(tc.tile_pool(name="const", bufs=1))
        # one PSUM pool, 8 banks total, multiplexed across phases:
        #   stA,stB: [128,2,512] double-bank (S tiles / qkv acc pairs / po)
        #   outA,outB: [128,512] single-bank (attention out accumulators)
        #   lb: [1,512] row-sum accumulator; rb: [128,512] 1/l broadcast
        psp = ctx.enter_context(tc.tile_pool(name="psp", bufs=1, space=PSUM))

        def ps2(i, name):        # 2-bank tile [128, 2, 512] f32
            return psp.tile([128, 2, 512], F32, tag=f"st{i % 2}", name=name)

        def ps1(tag, name, shape=(128, 512)):  # 1-bank tile f32
            return psp.tile(list(shape), F32, tag=tag, name=name)

        # constants
        tri = constp.tile([128, 128], F32, tag="tri", name="tri")
        nc.sync.dma_start(tri[:], tri_d.ap())
        idn = constp.tile([128, 128], BF16, tag="idn", name="idn")
        nc.sync.dma_start(idn[:], idn_d.ap())
        rott = constp.tile([DH, DH], BF16, tag="rot", name="rot")
        nc.sync.dma_start(rott[:], rot_d.ap())
        ones_f32 = constp.tile([128, 1], F32, tag="onef", name="onef")
        nc.vector.memset(ones_f32[:], 1.0)
        ones_col = constp.tile([128, 1], BF16, tag="onec", name="onec")
        nc.vector.tensor_copy(ones_col[:], ones_f32[:])
        cbias = constp.tile([128, 1], F32, tag="cbias", name="cbias")
        nc.vector.memset(cbias[:], -EXP_C)

        if reps > 1:
            rep_cm = tc.For_i(0, reps, 1)
            rep_cm.__enter__()

        with ExitStack() as kvctx:
            kvp = kvctx.enter_context(tc.tile_pool(name="kv", bufs=1))
            # resident K (d-major), V (token-major), Q (d-major) per batch
            k_t = [kvp.tile([128, S], BF16, tag=f"kt{b}", name=f"kt{b}")
                   for b in range(B)]
            v_sb = [kvp.tile([128, SKB, 128], BF16, tag=f"v{b}", name=f"v{b}")
                    for b in range(B)]
            q_all = [kvp.tile([128, HPC, S], BF16, tag=f"q{b}", name=f"q{b}")
                     for b in range(B)]

            # ============ phase 1: QKV + clip + RoPE (both batches) ========
            with ExitStack() as qctx:
                wqp = qctx.enter_context(tc.tile_pool(name="wq", bufs=1))
                slabp = qctx.enter_context(tc.tile_pool(name="slab", bufs=2))
                csp = qctx.enter_context(tc.tile_pool(name="cs", bufs=2))
                vtp = qctx.enter_context(tc.tile_pool(name="vt", bufs=1))
                workp = qctx.enter_context(tc.tile_pool(name="work", bufs=3))

                # resident qkv weights [128, KB, 768]: for ti=0, per-kb row
                # chunks so the first matmuls start ~2us in; rest in one DMA
                KB0 = 8   # chunked-prefix k-blocks
                w_sb = wqp.tile([128, KB, NF * 128], BF16, tag="w", name="w")

                def evict_feature(f, acc_ap, b, s0, cs_c, v_t):
                    """clip (+RoPE for q/k) of one feature's psum column."""
                    if f == NF - 1:  # v: clip only
                        nc.vector.tensor_scalar(
                            v_t[b][:, s0:s0 + TT], acc_ap,
                            -CLIP, CLIP, Alu.max, Alu.min,
                        )
                        return
                    cl = workp.tile([128, TT], BF16, tag="clip", name="clip")
                    nc.vector.tensor_scalar(
                        cl[:], acc_ap, -CLIP, CLIP, Alu.max, Alu.min
                    )
                    rps = ps1("out" + "AB"[f % 2], f"rot{f}")
                    nc.tensor.matmul(
                        rps[:], R(rott[:]), R(cl[:]), start=True, stop=True
                    )
                    t1 = workp.tile([128, TT], F32, tag="t1", name="t1")
                    nc.vector.tensor_tensor(
                        t1[:], cl[:], cs_c[:, 0, :], Alu.mult
                    )
                    t2 = workp.tile([128, TT], F32, tag="t2", name="t2")
                    nc.vector.tensor_tensor(
                        t2[:], rps[:], cs_c[:, 1, :], Alu.mult
                    )
                    dst = (q_all[b][:, f, s0:s0 + TT] if f < HPC
                           else k_t[b][:, s0:s0 + TT])
                    nc.vector.tensor_tensor(dst, t1[:], t2[:], Alu.add)

                v_t = [None] * B
                for ti in range(T // TT):
                    t0 = ti * TT
                    b = t0 // S
                    s0 = t0 - b * S
                    if s0 == 0:
                        v_t[b] = vtp.tile([128, S], BF16, tag=f"vt{b}",
                                          name=f"vt{b}")

                    ht = slabp.tile([128, KB, TT], BF16, tag="hid", name="hid")
                    if ti == 0:
                        # chunked DMAs: kb-row k of weights + hidden arrive
                        # together so the kb-ordered matmuls below can start
                        # as soon as the first chunks land
                        for kc in range(KB):
                            nc.sync.dma_start(
                                w_sb[:, kc, :],
                                wqkv.ap()[kc * 128:(kc + 1) * 128, :]
                                .rearrange("(kb p) f -> p kb f", p=128),
                            )
                            nc.sync.dma_start(
                                ht[:, kc, :],
                                hid.ap()[kc * 128:(kc + 1) * 128, t0:t0 + TT]
                                .rearrange("(kb p) c -> p kb c", p=128),
                            )
                    else:
                        nc.sync.dma_start(
                            ht[:],
                            hid.ap()[:, t0:t0 + TT].rearrange(
                                "(kb p) c -> p kb c", p=128
                            ),
                        )
                    cs_c = csp.tile([DH, 2, TT], F32, tag="cs", name="cs")
                    nc.sync.dma_start(
                        cs_c[:],
                        cs_d.ap()[:, :, t0:t0 + TT].rearrange(
                            "s p c -> p s c"
                        ),
                    )

                    if ti == 0:
                        # kb-ordered: all 6 features per kb chunk, three
                        # 2-feature psum pairs live (stA, stB, outA+outB)
                        accs = [ps2(0, "qkv0"), ps2(1, "qkv1")]
                        acc2 = [ps1("lb", "qkv4"), ps1("rb", "qkv5")]

                        def acc_ap(f):
                            return (accs[f // 2][:, f % 2, :] if f < 4
                                    else acc2[f - 4][:])

                        for kb in range(KB):
                            for f in range(NF):
                                nc.tensor.matmul(
                                    acc_ap(f),
                                    R(w_sb[:, kb, f * 128:(f + 1) * 128]),
                                    R(ht[:, kb, :]),
                                    start=(kb == 0),
                                    stop=(kb == KB - 1),
                                )
                        for f in range(NF):
                            evict_feature(f, acc_ap(f), b, s0, cs_c, v_t)
                    else:
                        # 3-way acc rotation (stA, stB, outA+outB) so the
                        # next tile's pair 0 never waits on this tile's
                        # trailing evictions
                        for fp in range(NF // 2):   # feature pairs
                            if fp < 2:
                                acc = ps2(fp, f"qkv{fp}")
                                aps = [acc[:, 0, :], acc[:, 1, :]]
                            else:
                                aps = [ps1("outA", "qkv4")[:],
                                       ps1("outB", "qkv5")[:]]
                            for half in range(2):
                                f = fp * 2 + half
                                for kb in range(KB):
                                    nc.tensor.matmul(
                                        aps[half],
                                        R(w_sb[:, kb, f * 128:(f + 1) * 128]),
                                        R(ht[:, kb, :]),
                                        start=(kb == 0),
                                        stop=(kb == KB - 1),
                                    )
                                # evict f while f+1 / next pair runs
                                evict_feature(
                                    f, aps[half], b, s0, cs_c, v_t
                                )

                    # V -> token-major via PE transpose at end of each batch
                    if s0 == S - TT:
                        for tg in range(SKB // 4):
                            tps = psp.tile([128, 4, 128], BF16,
                                           tag=("lb", "rb")[tg % 2],
                                           name="vtps")
                            for tj in range(4):
                                to = tg * 4 + tj
                                nc.tensor.transpose(
                                    R(tps[:, tj, :]),
                                    R(v_t[b][:, to * 128:(to + 1) * 128]),
                                    R(idn[:]),
                                )
                            nc.scalar.copy(
                                v_sb[b][:, tg * 4:(tg + 1) * 4, :], tps[:]
                            )

            # ============ phase 2: attention + out_proj (per batch) ========
            with ExitStack() as actx:
                attnp = actx.enter_context(tc.tile_pool(name="attn", bufs=1))
                ptp = actx.enter_context(tc.tile_pool(name="pt", bufs=3))
                normp = actx.enter_context(tc.tile_pool(name="norm", bufs=2))
                owp = actx.enter_context(tc.tile_pool(name="ow", bufs=1))
                oevp = actx.enter_context(tc.tile_pool(name="oev", bufs=4))

                ow_sb = owp.tile([128, HPC, D], BF16, tag="ow", name="ow")
                nc.sync.dma_start(
                    ow_sb[:], outw.ap().rearrange("(kb p) f -> p kb f", p=128)
                )

                for b in range(B):
                    at = attnp.tile([128, HPC, S], BF16, tag="attn",
                                    name="attn")

                    def n_pairs(qt):
                        n_kb = qt + 1 if causal else SKB
                        return (n_kb + 1) // 2

                    units = [(qt, pi) for qt in range(QT)
                             for pi in range(n_pairs(qt))]
                    qstate = {}   # qt -> (out_ps, l_ps)

                    def s_pair(idx):
                        """S matmuls + diag mask + exp for unit idx; returns
                        the P tile."""
                        qt, pi = units[idx]
                        n_kb = qt + 1 if causal else SKB
                        kb0 = pi * 2
                        nh = min(2, n_kb - kb0)
                        qmv = q_all[b][:, :, qt * 128:(qt + 1) * 128]
                        st = ps2(idx, "st")
                        for hf in range(nh):
                            kb = kb0 + hf
                            nc.tensor.matmul(
                                st[:, hf, :],
                                R(k_t[b][:, kb * 128:(kb + 1) * 128]),
                                R(qmv),
                                start=True,
                                stop=True,
                            )
                            if causal and kb == qt:
                                tri_b = _tri_bcast(tri)
                                if tri_b is not None:
                                    nc.vector.tensor_tensor(
                                        st[:, hf, :], st[:, hf, :],
                                        tri_b, Alu.add,
                                    )
                                else:
                                    for h in range(HPC):
                                        nc.vector.tensor_tensor(
                                            st[:, hf, h * 128:(h + 1) * 128],
                                            st[:, hf, h * 128:(h + 1) * 128],
                                            tri[:], Alu.add,
                                        )
                        pt = ptp.tile([128, 2, 512], BF16, tag="pt",
                                      name="pt")
                        nc.scalar.activation(
                            pt[:, 0:nh, :], st[:, 0:nh, :],
                            Exp, bias=cbias[:], scale=ISQ,
                        )
                        return pt

                    def pv_pair(idx, pt):
                        """PV + row-sum matmuls for unit idx; finish the qt
                        (normalize) after its last pair."""
                        qt, pi = units[idx]
                        n_kb = qt + 1 if causal else SKB
                        kb0 = pi * 2
                        nh = min(2, n_kb - kb0)
                        if pi == 0:
                            qstate[qt] = (
                                ps1("out" + "AB"[qt % 2], "outT"),
                                psp.tile([1, 512], F32, tag="lb", name="l"),
                            )
                        out_ps, l_ps = qstate[qt]
                        last = pi == n_pairs(qt) - 1
                        for hf in range(nh):
                            kb = kb0 + hf
                            nc.tensor.matmul(
                                out_ps[:],
                                R(v_sb[b][:, kb, :]),
                                R(pt[:, hf, :]),
                                start=(pi == 0 and hf == 0),
                                stop=(last and hf == nh - 1),
                                skip_group_check=True,
                            )
                            nc.tensor.matmul(
                                l_ps[:],
                                R(ones_col[:]),
                                R(pt[:, hf, :]),
                                start=(pi == 0 and hf == 0),
                                stop=(last and hf == nh - 1),
                                skip_group_check=True,
                            )
                        if not last:
                            return
                        # normalize: r = 1/l, broadcast over partitions on
                        # the (otherwise idle) GpSimd engine
                        r_sb = normp.tile([1, 512], F32, tag="rsb",
                                          name="rsb")
                        nc.vector.reciprocal_approx_fast(r_sb[:], l_ps[:])
                        rb_sb = normp.tile([128, 512], F32, tag="rbb",
                                           name="rbb")
                        nc.gpsimd.partition_broadcast(rb_sb[:], r_sb[:])
                        # free sizes match (4*128 == 512) and iteration order
                        # agrees, so no reshape is needed
                        nc.vector.tensor_tensor(
                            at[:, :, qt * 128:(qt + 1) * 128],
                            out_ps[:], rb_sb[:], Alu.mult,
                        )

                    # 2-deep software pipeline over all (qt, pair) units:
                    # S of units idx+1, idx+2 issue before PV/l of idx, so
                    # exp latency is hidden even across qt boundaries
                    pts = [s_pair(0)]
                    if len(units) > 1:
                        pts.append(s_pair(1))
                    for idx in range(len(units)):
                        if idx + 2 < len(units):
                            pts.append(s_pair(idx + 2))
                        pv_pair(idx, pts[idx])
                        pts[idx] = None

                    # ---- out_proj partial for this batch ----
                    for mi in range(S // 128):
                        m = b * (S // 128) + mi
                        ml = mi * 128
                        for pp in range(D // PW):
                            of0 = pp * PW
                            po = ps2(m * (D // PW) + pp, "po")
                            for kb in range(HPC):
                                for j in range(2):
                                    nc.tensor.matmul(
                                        po[:, j, :],
                                        R(at[:, kb, ml:ml + 128]),
                                        R(ow_sb[:, kb,
                                                of0 + j * 512:
                                                of0 + (j + 1) * 512]),
                                        start=(kb == 0),
                                        stop=(kb == HPC - 1),
                                        skip_group_check=True,
                                    )
                            oe = oevp.tile([128, PW], F32, tag="oe",
                                           name="oe")
                            nc.scalar.activation(oe[:], po[:], Copy)
                            nc.sync.dma_start(out_d.ap()[m, pp], oe[:])

        if reps > 1:
            rep_cm.__exit__(None, None, None)

    nc.compile()
    return nc


def rope_tables(position_ids, T):
    inv_freq = 1.0 / (
        ROPE_THETA ** (np.arange(0, DH, 2, dtype=np.float32) / DH)
    )
    freqs = (
        position_ids.astype(np.float32)[:, :, None] * inv_freq[None, None, :]
    )  # [B,S,64]
    emb = np.concatenate((freqs, freqs), axis=-1)  # [B,S,128]
    cos_t = np.ascontiguousarray(np.cos(emb).reshape(T, DH).T.astype(np.float32))
    sin_t = np.ascontiguousarray(np.sin(emb).reshape(T, DH).T.astype(np.float32))
    return cos_t, sin_t


def rot_matrix():
    """rotate_half as a matrix: rot(q) = R @ q for a [DH] head vector."""
    Rm = np.zeros((DH, DH), dtype=np.float32)
    half = DH // 2
    for d in range(half):
        Rm[d, d + half] = -1.0
        Rm[d + half, d] = 1.0
    return np.ascontiguousarray(Rm.T).astype(NP_BF16)  # lhsT for the PE


def tri_mask():
    tri = np.zeros((128, 128), dtype=np.float32)
    ki, qj = np.meshgrid(np.arange(128), np.arange(128), indexing="ij")
    tri[ki > qj] = NEG
    return tri


def make_host_inputs(hidden_states, position_ids, Wqkv_w, out_w, B, S, D):
    """Per-core input maps (host-side sharding / layout prep)."""
    T = B * S
    hid_t = np.ascontiguousarray(
        hidden_states.reshape(T, D).T.astype(NP_BF16)
    )
    cs_t = np.ascontiguousarray(np.stack(rope_tables(position_ids, T)))
    rot_t = rot_matrix()
    tri = tri_mask()
    idn = np.eye(128, dtype=NP_BF16)

    n_kv = D // 4  # KV_HEADS * HEAD_DIM
    in_maps = []
    for c in range(N_CORES):
        wq = Wqkv_w[c * HPC * DH:(c + 1) * HPC * DH]            # [512, D]
        wk = Wqkv_w[D + c * DH:D + (c + 1) * DH]                # [128, D]
        wv = Wqkv_w[D + n_kv + c * DH:D + n_kv + (c + 1) * DH]  # [128, D]
        wc = np.concatenate([wq, wk, wv], axis=0)               # [768, D]
        wc_t = np.ascontiguousarray(wc.T).astype(NP_BF16)       # [D, 768]
        ow_c = np.ascontiguousarray(
            out_w[:, c * HPC * DH:(c + 1) * HPC * DH].T
        ).astype(NP_BF16)  # [512, D]
        in_maps.append(
            {
                "hidden_t": hid_t,
                "wqkv_t": wc_t,
                "outw_t": ow_c,
                "cs_t": cs_t,
                "rot_t": rot_t,
                "trimask": tri,
                "identity": idn,
            }
        )
    return in_maps


_PROGRAM_CACHE = {}


def _get_program(B, S, D, causal):
    key = (B, S, D, causal)
    if key not in _PROGRAM_CACHE:
        _PROGRAM_CACHE[key] = build_program(B, S, D, causal=causal)
    return _PROGRAM_CACHE[key]


def _detect_causal(attention_mask, B, S):
    causal = np.triu(
        np.full((S, S), np.finfo(np.float32).min, dtype=np.float32), 1
    )
    am = np.asarray(attention_mask)
    if am.shape == (B, 1, S, S):
        if np.array_equal(am, np.broadcast_to(causal[None, None], (B, 1, S, S))):
            return True
        if not am.any():
            return False
    raise ValueError(
        "kernel only supports the causal mask from setup_inputs() or an "
        "all-zero mask"
    )


def kernel(hidden_states, position_ids, attention_mask, Wqkv_w, out_w):
    hidden_states = np.asarray(hidden_states)
    position_ids = np.asarray(position_ids)
    Wqkv_w = np.asarray(Wqkv_w)
    out_w = np.asarray(out_w)

    B, S, D = hidden_states.shape
    causal = _detect_causal(attention_mask, B, S)
    nc = _get_program(B, S, D, causal)
    in_maps = make_host_inputs(
        hidden_states, position_ids, Wqkv_w, out_w, B, S, D
    )
    res = run_bass_kernel_spmd(nc, in_maps, list(range(N_CORES)))
    out = res.results[0]["out_partial"].astype(np.float64)
    for c in range(1, N_CORES):
        out += res.results[c]["out_partial"]
    # out is [MT, OH, 128, EW] tiled; reassemble to [B, S, D]
    out = out.transpose(0, 2, 1, 3).reshape(B, S, D)
    return out.astype(np.float32)
